# revision 6
# baseline (speedup 1.0000x reference)
"""Trainium2 kernel for cellpose-style flow integration (grid_sample scan).

Strategy (v3 — stall-free per-column indirect gathers):
  - Host builds a padded "patch table" T[r*2050+c] = the 8 values
    [a00,a01,a10,a11,b00,b01,b10,b11] of the 2x2 bilinear corner patch at
    padded pixel (r,c), PRE-SCALED by 1024 so the integration state can be
    kept directly in padded pixel coordinates u = pt*1024 + 1024.5.
  - Points are sharded across 8 NeuronCores (32768 each, laid out [128,256]).
  - The HW indirect-DMA ucode gathers exactly 128 rows (one offset per
    partition) per instruction at ~550ns/instruction back-to-back, so an
    iteration needs 256 gather instructions; the Pool engine's descriptor
    generation is the bottleneck (~140us/iter).  Everything else (floor/
    index arithmetic on Act+DVE, bilinear lerp and state update on DVE)
    is pipelined underneath it with a chunked software pipeline.
  - floor(u) = i32(u - 0.5) using the round-to-nearest convert (verified
    on HW); the only miss is exact-integer u where ties-to-even may give
    floor-1 with fx = 1.0, which the lerp reproduces exactly.
"""
import numpy as np

H = W = 2048
NPTS = 262144
N_CORES = 8
PTS_PER_CORE = NPTS // N_CORES          # 32768
P = 128
F = PTS_PER_CORE // P                   # 256 free elems per partition
PAD = 2050                              # padded table row length
NCHUNK = 4
ACT_CONVERTS = True                     # dtype converts on Act engine vs DVE

_compiled = {}


def _build_nc(niter: int):
    import concourse.bass as bass
    import concourse.mybir as mybir
    import concourse.tile as tile
    from concourse import bacc

    f32 = mybir.dt.float32
    i32 = mybir.dt.int32
    Alu = mybir.AluOpType

    nc = bacc.Bacc("TRN2", target_bir_lowering=False, debug=False,
                   num_devices=N_CORES)
    tab = nc.dram_tensor("tab", [PAD * PAD, 8], f32, kind="ExternalInput").ap()
    p0x = nc.dram_tensor("p0x", [P, F], f32, kind="ExternalInput").ap()
    p0y = nc.dram_tensor("p0y", [P, F], f32, kind="ExternalInput").ap()
    outx = nc.dram_tensor("outx", [P, F], f32, kind="ExternalOutput").ap()
    outy = nc.dram_tensor("outy", [P, F], f32, kind="ExternalOutput").ap()

    FC = F // NCHUNK

    with tile.TileContext(nc) as tc:
        with (
            tc.tile_pool(name="state", bufs=1) as state,
            tc.tile_pool(name="scratch", bufs=NCHUNK + 1) as scratch,
            tc.tile_pool(name="gbuf", bufs=NCHUNK + 1) as gbuf,
        ):
            ux = state.tile([P, F], f32, tag="ux")
            uy = state.tile([P, F], f32, tag="uy")
            nc.sync.dma_start(out=ux[:], in_=p0x[:])
            nc.sync.dma_start(out=uy[:], in_=p0y[:])

            # per-chunk live tiles (written by coords, read by gather/lerp)
            fxs, fys, qis, gs = {}, {}, {}, {}

            def coords(c):
                """fx/fy = fract(u); qi = floor(uy)*2050 + floor(ux)."""
                cs = slice(c * FC, (c + 1) * FC)
                fx = scratch.tile([P, FC], f32, tag=f"fx{c}")
                fy = scratch.tile([P, FC], f32, tag=f"fy{c}")
                xf = scratch.tile([P, FC], f32, tag=f"xf{c}")
                yf = scratch.tile([P, FC], f32, tag=f"yf{c}")
                qf = scratch.tile([P, FC], f32, tag=f"qf{c}")
                qi = scratch.tile([P, FC], i32, tag=f"qi{c}")
                t = scratch.tile([P, FC], f32, tag=f"t{c}")
                ti = scratch.tile([P, FC], i32, tag=f"ti{c}")
                Copy = mybir.ActivationFunctionType.Copy
                for (u, fr, fl) in ((ux[:, cs], fx, xf), (uy[:, cs], fy, yf)):
                    if ACT_CONVERTS:
                        nc.scalar.activation(out=t[:], in_=u, func=Copy,
                                             bias=-0.5, scale=1.0)
                        nc.scalar.activation(out=ti[:], in_=t[:], func=Copy)
                        nc.scalar.activation(out=fl[:], in_=ti[:], func=Copy)
                    else:
                        nc.vector.tensor_scalar(out=t[:], in0=u, scalar1=0.5,
                                                scalar2=None, op0=Alu.subtract)
                        nc.vector.tensor_copy(out=ti[:], in_=t[:])
                        nc.vector.tensor_copy(out=fl[:], in_=ti[:])
                    nc.vector.tensor_tensor(out=fr[:], in0=u, in1=fl[:],
                                            op=Alu.subtract)
                nc.vector.scalar_tensor_tensor(out=qf[:], in0=yf[:],
                                               scalar=2050.0, in1=xf[:],
                                               op0=Alu.mult, op1=Alu.add)
                if ACT_CONVERTS:
                    nc.scalar.copy(out=qi[:], in_=qf[:])
                else:
                    nc.vector.tensor_copy(out=qi[:], in_=qf[:])
                fxs[c], fys[c], qis[c] = fx, fy, qi

            def gather(c):
                """FC per-column indirect gathers: 128 patches each."""
                g = gbuf.tile([P, FC, 8], f32, tag=f"g{c}")
                qi = qis[c]
                for j in range(FC):
                    nc.gpsimd.indirect_dma_start(
                        out=g[:, j, :],
                        out_offset=None,
                        in_=tab[:, :],
                        in_offset=bass.IndirectOffsetOnAxis(
                            ap=qi[:, j:j + 1], axis=0),
                    )
                gs[c] = g

            def lerp_update(c):
                cs = slice(c * FC, (c + 1) * FC)
                g, fx, fy = gs[c], fxs[c], fys[c]
                d = scratch.tile([P, FC, 4], f32, tag=f"d{c}")
                h = scratch.tile([P, FC, 4], f32, tag=f"h{c}")
                nc.vector.tensor_tensor(out=d[:], in0=g[:, :, 1::2],
                                        in1=g[:, :, 0::2], op=Alu.subtract)
                nc.vector.tensor_tensor(out=d[:], in0=d[:],
                                        in1=fx[:].to_broadcast([P, FC, 4]),
                                        op=Alu.mult)
                nc.vector.tensor_tensor(out=h[:], in0=g[:, :, 0::2],
                                        in1=d[:], op=Alu.add)
                d2 = scratch.tile([P, FC, 2], f32, tag=f"d2{c}")
                s = scratch.tile([P, FC, 2], f32, tag=f"s{c}")
                nc.vector.tensor_tensor(out=d2[:], in0=h[:, :, 1::2],
                                        in1=h[:, :, 0::2], op=Alu.subtract)
                nc.vector.tensor_tensor(out=d2[:], in0=d2[:],
                                        in1=fy[:].to_broadcast([P, FC, 2]),
                                        op=Alu.mult)
                nc.vector.tensor_tensor(out=s[:], in0=h[:, :, 0::2],
                                        in1=d2[:], op=Alu.add)
                # u += s ; clamp to [0.5, 2048.5]
                for (u, k) in ((ux[:, cs], 0), (uy[:, cs], 1)):
                    nc.vector.tensor_tensor(out=u, in0=u, in1=s[:, :, k],
                                            op=Alu.add)
                    nc.vector.tensor_scalar(out=u, in0=u, scalar1=0.5,
                                            scalar2=2048.5, op0=Alu.max,
                                            op1=Alu.min)

            # software pipeline: fill, then per chunk lerp -> coords -> gather
            for c in range(NCHUNK):
                coords(c)
                gather(c)
            for it in range(niter):
                for c in range(NCHUNK):
                    lerp_update(c)
                    if it + 1 < niter:
                        coords(c)
                        gather(c)

            # final: pix = ((u - 1024.5) / 1024 + 1) * 1023.5
            ox = state.tile([P, F], f32, tag="ox")
            oy = state.tile([P, F], f32, tag="oy")
            for (u, o) in ((ux, ox), (uy, oy)):
                nc.vector.tensor_scalar(out=o[:], in0=u[:], scalar1=1024.5,
                                        scalar2=1.0 / 1024.0,
                                        op0=Alu.subtract, op1=Alu.mult)
                nc.vector.tensor_scalar(out=o[:], in0=o[:], scalar1=1.0,
                                        scalar2=1023.5, op0=Alu.add,
                                        op1=Alu.mult)
            nc.sync.dma_start(out=outx[:], in_=ox[:])
            nc.sync.dma_start(out=outy[:], in_=oy[:])

    nc.compile()
    return nc


def _build_table(dP: np.ndarray) -> np.ndarray:
    """T[r*2050+c, 0:8] = 2x2 patch of (im0,im1)*1024 at padded (r,c)."""
    scale = np.float32(2.0 / 2047.0)
    im0 = (dP[1] * scale).astype(np.float32) * np.float32(1024.0)  # adds to x
    im1 = (dP[0] * scale).astype(np.float32) * np.float32(1024.0)  # adds to y
    imp = np.zeros((PAD + 1, PAD + 1, 2), np.float32)
    imp[1:H + 1, 1:W + 1, 0] = im0
    imp[1:H + 1, 1:W + 1, 1] = im1
    T = np.empty((PAD, PAD, 8), np.float32)
    T[:, :, 0] = imp[:PAD, :PAD, 0]       # a00
    T[:, :, 1] = imp[:PAD, 1:, 0]         # a01
    T[:, :, 2] = imp[1:, :PAD, 0]         # a10
    T[:, :, 3] = imp[1:, 1:, 0]           # a11
    T[:, :, 4] = imp[:PAD, :PAD, 1]       # b00
    T[:, :, 5] = imp[:PAD, 1:, 1]         # b01
    T[:, :, 6] = imp[1:, :PAD, 1]         # b10
    T[:, :, 7] = imp[1:, 1:, 1]           # b11
    return T.reshape(PAD * PAD, 8)


def _initial_pts(inds: np.ndarray):
    """Initial padded pixel coords u = pt*1024 + 1024.5, pt in [-1,1]."""
    f = np.float32
    sizes = f(2047.0)
    ptx = inds[1].astype(f) / sizes * f(2.0) - f(1.0)
    pty = inds[0].astype(f) / sizes * f(2.0) - f(1.0)
    ux = ptx * f(1024.0) + f(1024.5)
    uy = pty * f(1024.0) + f(1024.5)
    return ux, uy


def kernel(dP: np.ndarray, inds: np.ndarray, niter) -> np.ndarray:
    from concourse.bass_utils import run_bass_kernel_spmd

    niter = int(niter)
    dP = np.asarray(dP, np.float32)
    inds = np.asarray(inds)

    if niter not in _compiled:
        _compiled[niter] = _build_nc(niter)
    nc = _compiled[niter]

    T = _build_table(dP)
    ptx, pty = _initial_pts(inds)

    in_maps = []
    for i in range(N_CORES):
        sl = slice(i * PTS_PER_CORE, (i + 1) * PTS_PER_CORE)
        in_maps.append({
            "tab": T,
            "p0x": ptx[sl].reshape(P, F),
            "p0y": pty[sl].reshape(P, F),
        })

    res = run_bass_kernel_spmd(nc, in_maps, list(range(N_CORES)))

    out = np.empty((2, NPTS), np.float32)
    for i in range(N_CORES):
        sl = slice(i * PTS_PER_CORE, (i + 1) * PTS_PER_CORE)
        out[0, sl] = res.results[i]["outy"].reshape(-1)
        out[1, sl] = res.results[i]["outx"].reshape(-1)
    return out
